# revision 10
# baseline (speedup 1.0000x reference)
"""Trainium2 Bass kernel for DPMultiheadAttention (L=2048, B=2, E=1024, H=16).

Sharding: batch*head parallel across 8 cores. Core c handles batch c%2 and
heads [4*(c//2), 4*(c//2)+4). Each core computes q/k/v projections for its
256-feature slice, per-head attention, and a partial out-projection; the host
sums the per-batch partials.

v2 design (single fused pipeline, no phase barriers):
  - Scores are row-tiled on the PE: each head's contraction is only 64 dims,
    so the two heads of a pair run CONCURRENTLY in disjoint 64-row groups of
    the 128x128 array (lhsT/rhs base partitions 0 and 64) -> ~2x on scores.
    Q^T/K^T are stored compactly (even head on partitions 0-63, odd head on
    64-127) with no zero padding.
  - The softmax denominators ride as a ones-column in the padded V operand of
    the context matmul (V layout [P, lt, h, 64+1+63]).
  - One shared PSUM pool ([128,1024] fp32 x2 bufs, 4 banks) serves projection
    tiles, score tiles and out-projection tiles; ctx accumulators take the
    other 4 banks. Emission order is a static schedule that interleaves
    projection/V/out-proj matmuls into the exp-paced attention stream so the
    PE never idles while the ACT engine (the co-bottleneck, ~149us of exp)
    runs continuously from ~10us onward.
  - DMAs are token-sliced (quarter tensors) and priority-ordered so the first
    score matmuls start ~10us in instead of waiting for full 4MB tensors.
  - Normalization: reciprocal_approx_fast on the sums row (18-bit accuracy),
    broadcast down 64 partitions with a step-0-source DMA, one multiply into
    bf16 ctx. PSUM evac of raw ctx on GpSimd; out-proj bias+evac on GpSimd;
    q/k/v bias evacs on DVE. Output is bf16 (host accumulates in fp32).
"""

import numpy as np

import concourse.bass as bass
import concourse.tile as tile
from concourse import mybir
from concourse.bass_utils import run_bass_kernel_spmd

L = 2048
B = 2
E = 1024
H = 16
D = 64
NCORES = 8
HPC = H // NCORES * B  # heads per core = 4
FL = HPC * D  # local feature slice = 256
P = 128

BF16 = mybir.dt.bfloat16
FP32 = mybir.dt.float32

TRACE = False
TRACE_KWARGS = {}
LAST_RESULTS = None


class PatchedTileContext(tile.TileContext):
    """This walrus build caps sync-wait slots per instruction at one; Tile's
    sem assigner freely attaches several. Split extra waits onto same-engine
    nops inserted just before the owning instruction."""

    MAX_WAITS = 1

    def _split_inst_waits(self, inst, out_list):
        si = getattr(inst, "sync_info", None)
        if si is not None and len(si.on_wait) > self.MAX_WAITS:
            waits = list(si.on_wait)
            keep = len(waits) - self.MAX_WAITS
            for i in range(0, keep, self.MAX_WAITS):
                out_list.append(
                    mybir.InstNoOp(
                        name=f"I-ws-{self.nc.next_id()}",
                        engine=inst.engine,
                        bass_nofuse=True,
                        sync_info=mybir.SyncInfo(
                            on_wait=waits[i : i + self.MAX_WAITS], on_update=[]
                        ),
                    )
                )
            inst.sync_info = mybir.SyncInfo(
                on_wait=waits[keep:], on_update=list(si.on_update)
            )
        out_list.append(inst)

    def _lower_ordered_insts(self, ordered):
        for insts in ordered.values():
            new_list = []
            for inst in insts:
                self._split_inst_waits(inst, new_list)
            insts[:] = new_list
        super()._lower_ordered_insts(ordered)

    def _drain_and_barrier(self, tick_clock, wait_clock):
        from bass_rust import SyncInfo
        from concourse.vector_clock import ScopedClock

        drain_inst = self.nc.sync.drain()
        wait_clock.add_sem_waits(
            drain_inst.ins, ScopedClock({None: tick_clock.global_clock})
        )
        si = drain_inst.ins.sync_info
        if si is not None and len(si.on_wait) > self.MAX_WAITS:
            waits = list(si.on_wait)
            drain_inst.ins.sync_info = SyncInfo(
                on_wait=waits[: self.MAX_WAITS], on_update=list(si.on_update)
            )
            for i in range(self.MAX_WAITS, len(waits), self.MAX_WAITS):
                nop = self.nc.sync.nop(nofuse=True)
                nop.ins.sync_info = SyncInfo(
                    on_wait=waits[i : i + self.MAX_WAITS], on_update=[]
                )

        self.nc.all_engine_barrier()
        assert self.sems is not None
        popped = self.nc._tile_sem_poison_stack.pop()
        assert popped is self._sem_poison
        self.nc.clear_and_free_semaphores(list(self.sems.allocated().values()))
        self.nc.all_engine_barrier()


def _ap3(ap, dims):
    return bass.AP(tensor=ap.tensor, offset=ap.offset, ap=dims)


def _bcast_ap(t):
    """DRAM 1-D tensor -> (128, len) partition-broadcast AP for DMA."""
    ap = t[:]
    return bass.AP(tensor=ap.tensor, offset=ap.offset, ap=[[0, P], *ap.ap])


KT = E // P  # 8 contraction tiles for projections
MT = FL // P  # 2 feature tiles (head pairs)
LT = L // P  # 16 token tiles of 128


def build_nc():
    nc = bass.Bass()

    xq = nc.declare_dram_parameter("xq_t", [E, L], BF16, isOutput=False)
    xk = nc.declare_dram_parameter("xk_t", [E, L], BF16, isOutput=False)
    xv = nc.declare_dram_parameter("xv_t", [E, L], BF16, isOutput=False)
    wq = nc.declare_dram_parameter("wq_t", [E, FL], BF16, isOutput=False)
    wk = nc.declare_dram_parameter("wk_t", [E, FL], BF16, isOutput=False)
    wv = nc.declare_dram_parameter("wv_t", [E, FL], BF16, isOutput=False)
    wo = nc.declare_dram_parameter("wo_t", [FL, E], BF16, isOutput=False)
    bq = nc.declare_dram_parameter("bq", [FL], FP32, isOutput=False)
    bk = nc.declare_dram_parameter("bk", [FL], FP32, isOutput=False)
    bv = nc.declare_dram_parameter("bv", [FL], FP32, isOutput=False)
    out = nc.declare_dram_parameter("out_p", [L, E], BF16, isOutput=True)

    with PatchedTileContext(nc) as tc:
        with (
            tc.tile_pool(name="singles", bufs=1) as singles,
            tc.tile_pool(name="pt", bufs=6) as pt_pool,
            tc.tile_pool(name="norm", bufs=1) as norm_pool,
            tc.tile_pool(name="outsb", bufs=4) as out_pool,
            tc.tile_pool(name="spool", bufs=2, space="PSUM") as spool,
            tc.tile_pool(name="cpool", bufs=2, space="PSUM") as cpool,
        ):
            # ---- persistent SBUF ----
            wq_sb = singles.tile([P, KT, FL], BF16, tag="wq")
            wk_sb = singles.tile([P, KT, FL], BF16, tag="wk")
            wv_sb = singles.tile([P, KT, FL], BF16, tag="wv")
            wo_sb = singles.tile([P, MT, E], BF16, tag="wo")
            bq_sb = singles.tile([P, MT], FP32, tag="bq")
            bk_sb = singles.tile([P, MT], FP32, tag="bk")
            bv_sb = singles.tile([P, FL], FP32, tag="bv")
            qt_sb = singles.tile([P, MT, L], BF16, tag="qt")
            kt_sb = singles.tile([P, MT, L], BF16, tag="kt")
            ctx_sb = singles.tile([P, MT, L], BF16, tag="ctx")
            v_sb = singles.tile([P, LT, HPC, P], BF16, tag="v")
            xq_sb = singles.tile([P, KT, L], BF16, tag="xq")
            xk_sb = singles.tile([P, KT, L], BF16, tag="xk")
            xv_sb = singles.tile([P, KT, L], BF16, tag="xv")

            xq_re = xq.rearrange("(o p) m -> p o m", p=P)
            xk_re = xk.rearrange("(o p) m -> p o m", p=P)
            xv_re = xv.rearrange("(o p) m -> p o m", p=P)

            # ---- DMAs, priority ordered; big activations token-quartered.
            # Weights/biases go on the scalar+vector queues (idle at start),
            # activations on sync and gpsimd queues.
            nc.scalar.dma_start(wq_sb[:], wq.rearrange("(o p) f -> p o f", p=P))
            nc.sync.dma_start(xq_sb[:, :, 0:512], xq_re[:, :, 0:512])
            nc.scalar.dma_start(wk_sb[:], wk.rearrange("(o p) f -> p o f", p=P))
            nc.gpsimd.dma_start(xk_sb[:, :, 0:512], xk_re[:, :, 0:512])
            nc.scalar.dma_start(bq_sb[:], bq.rearrange("(o p) -> p o", p=P))
            nc.scalar.dma_start(bk_sb[:], bk.rearrange("(o p) -> p o", p=P))
            nc.sync.dma_start(xq_sb[:, :, 512:1024], xq_re[:, :, 512:1024])
            nc.scalar.dma_start(wv_sb[:], wv.rearrange("(o p) f -> p o f", p=P))
            nc.gpsimd.dma_start(xv_sb[:, :, 0:512], xv_re[:, :, 0:512])
            nc.sync.dma_start(xk_sb[:, :, 512:1024], xk_re[:, :, 512:1024])
            nc.gpsimd.dma_start(xv_sb[:, :, 512:1024], xv_re[:, :, 512:1024])
            nc.sync.dma_start(xq_sb[:, :, 1024:2048], xq_re[:, :, 1024:2048])
            nc.gpsimd.dma_start(xv_sb[:, :, 1024:2048], xv_re[:, :, 1024:2048])
            nc.sync.dma_start(xk_sb[:, :, 1024:2048], xk_re[:, :, 1024:2048])
            nc.scalar.dma_start(wo_sb[:], wo.rearrange("(o p) f -> p o f", p=P))
            nc.scalar.dma_start(bv_sb[:], _bcast_ap(bv))

            # V padded layout: [V_h (64) | ones | zeros(63)]
            nc.vector.memset(v_sb[:], 0.0)
            nc.vector.memset(v_sb[:, :, :, D], 1.0)

            # ---------------- emission helpers ----------------
            def proj_quarter(x_sb, w_sb, o_sb, b_sb, mt, nq, ps):
                """One projection quarter: features [128*mt,+128) x tokens
                [512*nq,+512) -> o_sb[:, mt, 512*nq:+512]. ps: [P,1024] psum
                tile; uses column half nq%2."""
                col = (nq % 2) * 512
                for k in range(KT):
                    nc.tensor.matmul(
                        ps[:, col : col + 512],
                        w_sb[:, k, bass.ts(mt, P)],
                        x_sb[:, k, bass.ts(nq, 512)],
                        start=(k == 0),
                        stop=(k == KT - 1),
                    )
                nc.vector.tensor_scalar_add(
                    o_sb[:, mt, bass.ts(nq, 512)],
                    ps[:, col : col + 512],
                    b_sb[:, mt : mt + 1],
                )

            def proj_half(x_sb, w_sb, o_sb, b_sb, mt, nqp, name):
                """Two projection quarters sharing one [P,1024] psum tile."""
                ps = spool.tile([P, 1024], FP32, tag="ps", name=f"pp_{name}")
                proj_quarter(x_sb, w_sb, o_sb, b_sb, mt, 2 * nqp, ps)
                proj_quarter(x_sb, w_sb, o_sb, b_sb, mt, 2 * nqp + 1, ps)

            def v_group(g):
                """V projection for token tiles lt=4g..4g+3 (one psum tile)."""
                ps = spool.tile([P, 1024], FP32, tag="ps", name=f"vg_{g}")
                for li in range(4):
                    lt = 4 * g + li
                    for k in range(KT):
                        nc.tensor.matmul(
                            ps[:, li * 256 : li * 256 + 256],
                            xv_sb[:, k, bass.ts(lt, P)],
                            wv_sb[:, k, :],
                            start=(k == 0),
                            stop=(k == KT - 1),
                        )
                for li in range(4):
                    lt = 4 * g + li
                    nc.vector.tensor_add(
                        v_sb[:, lt, :, 0:D],
                        ps[:, li * 256 : li * 256 + 256].rearrange(
                            "p (h d) -> p h d", d=D
                        ),
                        bv_sb.rearrange("p (h d) -> p h d", d=D),
                    )

            def c_tile(lt):
                """Out-projection for token tile lt + bias evac + store."""
                ps = spool.tile([P, 1024], FP32, tag="ps", name=f"ot_{lt}")
                for kt in range(MT):
                    for nn in range(2):
                        nc.tensor.matmul(
                            ps[:, bass.ts(nn, 512)],
                            ctx_sb[:, kt, bass.ts(lt, P)],
                            wo_sb[:, kt, bass.ts(nn, 512)],
                            start=(kt == 0),
                            stop=(kt == MT - 1),
                        )
                osb = out_pool.tile([P, E], BF16, tag="osb", name=f"osb_{lt}")
                nc.vector.tensor_copy(osb[:], ps[:])
                nc.sync.dma_start(out[bass.ts(lt, P), :], osb[:])

            # ---------------- attention section ----------------
            def b_section(pair, qh, fill):
                """Attention for head pair `pair` on query half `qh` (1024 q).
                `fill`: dict j -> list of thunks emitted after scores+exp of
                iteration j (PE filler work scheduled into exp-paced slack).
                ctx matmuls lag scores by 2 iterations (pts pool depth 6)."""
                cps = [
                    cpool.tile([P, 1024], FP32, tag="c", name=f"c_{pair}_{qh}_{h}")
                    for h in range(2)
                ]
                pts = {}

                def scores_exp(j):
                    sps = []
                    for hh in range(2):
                        sps.append(
                            spool.tile(
                                [P, 1024], FP32, tag="ps",
                                name=f"s_{pair}_{qh}_{j}_{hh}",
                            )
                        )
                    for nn in range(2):
                        for hh in range(2):
                            r0 = D * hh
                            nc.tensor.matmul(
                                sps[hh][:, bass.ts(nn, 512)],
                                kt_sb[r0 : r0 + D, pair, bass.ts(j, P)],
                                qt_sb[
                                    r0 : r0 + D, pair,
                                    bass.ds(qh * 1024 + nn * 512, 512),
                                ],
                                start=True,
                                stop=True,
                            )
                    pts[j] = []
                    for hh in range(2):
                        pt = pt_pool.tile(
                            [P, 1024], BF16, tag="pt", name=f"pt_{pair}_{qh}_{j}_{hh}"
                        )
                        nc.scalar.activation(
                            pt[:], sps[hh][:], mybir.ActivationFunctionType.Exp
                        )
                        pts[j].append(pt)

                def ctx_mm(j):
                    for hh in range(2):
                        for nn in range(2):
                            nc.tensor.matmul(
                                cps[hh][:, bass.ts(nn, 512)],
                                v_sb[:, j, 2 * pair + hh, :],
                                pts[j][hh][:, bass.ts(nn, 512)],
                                start=(j == 0),
                                stop=(j == LT - 1),
                            )
                    del pts[j]

                LAG = 2
                for j in range(LT):
                    scores_exp(j)
                    for thunk in fill.get(j, ()):
                        thunk()
                    if j >= LAG:
                        ctx_mm(j - LAG)
                for j in range(LT - LAG, LT):
                    ctx_mm(j)

                # normalization: recip of sums row, broadcast, multiply
                for hh in range(2):
                    craw = norm_pool.tile(
                        [D + 1, 1024], FP32, tag="craw", name=f"cr_{pair}_{qh}_{hh}"
                    )
                    nc.vector.tensor_copy(craw[:], cps[hh][0 : D + 1, :])
                    # reciprocal spread over four 32-aligned partitions (a
                    # 1-partition reciprocal is ~6.5 DVE-cycles/element)
                    rt = norm_pool.tile([97, 256], FP32, tag="rt")
                    for k4 in range(4):
                        nc.vector.tensor_copy(
                            rt[32 * k4 : 32 * k4 + 1, :],
                            craw[D : D + 1, bass.ts(k4, 256)],
                        )
                    nc.vector.reciprocal(rt[:], rt[:])
                    rrow = norm_pool.tile([1, 1024], FP32, tag="rrow")
                    for k4 in range(4):
                        nc.vector.tensor_copy(
                            rrow[0:1, bass.ts(k4, 256)],
                            rt[32 * k4 : 32 * k4 + 1, :],
                        )
                    rb = norm_pool.tile([D, 1024], FP32, tag="rb")
                    rap = rrow[0:1, :]
                    nc.sync.dma_start(
                        out=_ap3(rb[:], [rb[:].ap[0], [1, 1], rb[:].ap[1]]),
                        in_=_ap3(rap, [[1, 1], [0, D], rap.ap[-1]]),
                    )
                    nc.vector.tensor_mul(
                        ctx_sb[D * hh : D * hh + D, pair, bass.ts(qh, 1024)],
                        craw[0:D, :],
                        rb[:],
                    )

            # ---------------- static schedule ----------------
            # Bootstrap: Q then K for pair 0, first query/key half.
            proj_half(xq_sb, wq_sb, qt_sb, bq_sb, 0, 0, "q00")
            proj_half(xk_sb, wk_sb, kt_sb, bk_sb, 0, 0, "k00")

            # (p0,qh0): needs kt mt0 nqp1 from j=8; v groups as ctx catches up;
            # Q mt0 nqp1 for qh1.
            b_section(0, 0, {
                1: [lambda: v_group(0)],
                4: [lambda: proj_half(xk_sb, wk_sb, kt_sb, bk_sb, 0, 1, "k01")],
                6: [lambda: v_group(1)],
                8: [lambda: proj_half(xq_sb, wq_sb, qt_sb, bq_sb, 0, 1, "q01")],
                10: [lambda: v_group(2)],
                13: [lambda: v_group(3)],
            })
            # (p0,qh1): fill with pair-1 projections.
            b_section(0, 1, {
                1: [lambda: proj_half(xk_sb, wk_sb, kt_sb, bk_sb, 1, 0, "k10")],
                5: [lambda: proj_half(xk_sb, wk_sb, kt_sb, bk_sb, 1, 1, "k11")],
                9: [lambda: proj_half(xq_sb, wq_sb, qt_sb, bq_sb, 1, 0, "q10")],
                13: [lambda: proj_half(xq_sb, wq_sb, qt_sb, bq_sb, 1, 1, "q11")],
            })
            b_section(1, 0, {})
            # (p1,qh1): fill with out-projection for the first query half.
            b_section(1, 1, {j: [lambda lt=j - 2: c_tile(lt)] for j in range(2, 10)})
            # tail: out-projection for the second query half.
            for lt in range(8, LT):
                c_tile(lt)

    return nc


_NC = None


def _get_nc():
    global _NC
    if _NC is None:
        _NC = build_nc()
    return _NC


def kernel(query, key, value, w_in, b_in, w_out, b_out):
    import ml_dtypes

    bf16 = ml_dtypes.bfloat16
    query = np.asarray(query, dtype=np.float32)
    key = np.asarray(key, dtype=np.float32)
    value = np.asarray(value, dtype=np.float32)
    w_in = np.asarray(w_in, dtype=np.float32)
    b_in = np.asarray(b_in, dtype=np.float32)
    w_out = np.asarray(w_out, dtype=np.float32)
    b_out = np.asarray(b_out, dtype=np.float32)

    scale = float(D) ** -0.5
    in_maps = []
    for c in range(NCORES):
        b = c % 2
        g = c // 2
        sl = slice(FL * g, FL * (g + 1))
        wq = w_in[0 * E : 1 * E][sl] * scale  # (256, 1024)
        wk = w_in[1 * E : 2 * E][sl]
        wv = w_in[2 * E : 3 * E][sl]
        in_maps.append(
            {
                "xq_t": np.ascontiguousarray(query[:, b, :].T).astype(bf16),
                "xk_t": np.ascontiguousarray(key[:, b, :].T).astype(bf16),
                "xv_t": np.ascontiguousarray(value[:, b, :].T).astype(bf16),
                "wq_t": np.ascontiguousarray(wq.T).astype(bf16),
                "wk_t": np.ascontiguousarray(wk.T).astype(bf16),
                "wv_t": np.ascontiguousarray(wv.T).astype(bf16),
                "wo_t": np.ascontiguousarray(w_out[:, sl].T).astype(bf16),
                "bq": np.ascontiguousarray(b_in[0 * E : 1 * E][sl] * scale),
                "bk": np.ascontiguousarray(b_in[1 * E : 2 * E][sl]),
                "bv": np.ascontiguousarray(b_in[2 * E : 3 * E][sl]),
            }
        )

    nc = _get_nc()
    res = run_bass_kernel_spmd(
        nc, in_maps, list(range(NCORES)), trace=TRACE, **TRACE_KWARGS
    )
    global LAST_RESULTS
    LAST_RESULTS = res

    out = np.zeros((L, B, E), dtype=np.float32)
    for c in range(NCORES):
        out[:, c % 2, :] += res.results[c]["out_p"].astype(np.float32)
    out += b_out
    return out


# revision 19
# speedup vs baseline: 1.0302x; 1.0302x over previous
"""Trainium2 Bass kernel for DPMultiheadAttention (L=2048, B=2, E=1024, H=16).

Sharding: batch*head parallel across 8 cores. Core c handles batch c%2 and
heads [4*(c//2), 4*(c//2)+4). Each core computes q/k/v projections for its
256-feature slice, per-head attention, and a partial out-projection; the host
sums the per-batch partials.

v2 design (single fused pipeline, no phase barriers):
  - Scores are row-tiled on the PE: each head's contraction is only 64 dims,
    so the two heads of a pair run CONCURRENTLY in disjoint 64-row groups of
    the 128x128 array (lhsT/rhs base partitions 0 and 64) -> ~2x on scores.
    Q^T/K^T are stored compactly (even head on partitions 0-63, odd head on
    64-127) with no zero padding.
  - The softmax denominators ride as a ones-column in the padded V operand of
    the context matmul (V layout [P, lt, h, 64+1+63]).
  - One shared PSUM pool ([128,1024] fp32 x2 bufs, 4 banks) serves projection
    tiles, score tiles and out-projection tiles; ctx accumulators take the
    other 4 banks. Emission order is a static schedule that interleaves
    projection/V/out-proj matmuls into the exp-paced attention stream so the
    PE never idles while the ACT engine (the co-bottleneck, ~149us of exp)
    runs continuously from ~10us onward.
  - DMAs are token-sliced (quarter tensors) and priority-ordered so the first
    score matmuls start ~10us in instead of waiting for full 4MB tensors.
  - Normalization: reciprocal_approx_fast on the sums row (18-bit accuracy),
    broadcast down 64 partitions with a step-0-source DMA, one multiply into
    bf16 ctx. PSUM evac of raw ctx on GpSimd; out-proj bias+evac on GpSimd;
    q/k/v bias evacs on DVE. Output is bf16 (host accumulates in fp32).
"""

import numpy as np

import concourse.bass as bass
import concourse.tile as tile
from concourse import mybir
from concourse.bass_utils import run_bass_kernel_spmd

L = 2048
B = 2
E = 1024
H = 16
D = 64
NCORES = 8
HPC = H // NCORES * B  # heads per core = 4
FL = HPC * D  # local feature slice = 256
P = 128

BF16 = mybir.dt.bfloat16
FP32 = mybir.dt.float32

TRACE = False
TRACE_KWARGS = {}
LAST_RESULTS = None
DEBUG_DUMP = False


class PatchedTileContext(tile.TileContext):
    """This walrus build caps sync-wait slots per instruction at one; Tile's
    sem assigner freely attaches several. Split extra waits onto same-engine
    nops inserted just before the owning instruction."""

    MAX_WAITS = 1

    def _split_inst_waits(self, inst, out_list):
        si = getattr(inst, "sync_info", None)
        if si is not None and len(si.on_wait) > self.MAX_WAITS:
            waits = list(si.on_wait)
            keep = len(waits) - self.MAX_WAITS
            for i in range(0, keep, self.MAX_WAITS):
                out_list.append(
                    mybir.InstNoOp(
                        name=f"I-ws-{self.nc.next_id()}",
                        engine=inst.engine,
                        bass_nofuse=True,
                        sync_info=mybir.SyncInfo(
                            on_wait=waits[i : i + self.MAX_WAITS], on_update=[]
                        ),
                    )
                )
            inst.sync_info = mybir.SyncInfo(
                on_wait=waits[keep:], on_update=list(si.on_update)
            )
        out_list.append(inst)

    def _lower_ordered_insts(self, ordered):
        for insts in ordered.values():
            new_list = []
            for inst in insts:
                self._split_inst_waits(inst, new_list)
            insts[:] = new_list
        super()._lower_ordered_insts(ordered)

    def _drain_and_barrier(self, tick_clock, wait_clock):
        from bass_rust import SyncInfo
        from concourse.vector_clock import ScopedClock

        drain_inst = self.nc.sync.drain()
        wait_clock.add_sem_waits(
            drain_inst.ins, ScopedClock({None: tick_clock.global_clock})
        )
        si = drain_inst.ins.sync_info
        if si is not None and len(si.on_wait) > self.MAX_WAITS:
            waits = list(si.on_wait)
            drain_inst.ins.sync_info = SyncInfo(
                on_wait=waits[: self.MAX_WAITS], on_update=list(si.on_update)
            )
            for i in range(self.MAX_WAITS, len(waits), self.MAX_WAITS):
                nop = self.nc.sync.nop(nofuse=True)
                nop.ins.sync_info = SyncInfo(
                    on_wait=waits[i : i + self.MAX_WAITS], on_update=[]
                )

        self.nc.all_engine_barrier()
        assert self.sems is not None
        popped = self.nc._tile_sem_poison_stack.pop()
        assert popped is self._sem_poison
        self.nc.clear_and_free_semaphores(list(self.sems.allocated().values()))
        self.nc.all_engine_barrier()


def _ap3(ap, dims):
    return bass.AP(tensor=ap.tensor, offset=ap.offset, ap=dims)


def _bcast_ap(t):
    """DRAM 1-D tensor -> (128, len) partition-broadcast AP for DMA."""
    ap = t[:]
    return bass.AP(tensor=ap.tensor, offset=ap.offset, ap=[[0, P], *ap.ap])


KT = E // P  # 8 contraction tiles for projections
MT = FL // P  # 2 feature tiles (head pairs)
LT = L // P  # 16 token tiles of 128


def build_nc():
    nc = bass.Bass()

    xq = nc.declare_dram_parameter("xq_t", [E, L], BF16, isOutput=False)
    xk = nc.declare_dram_parameter("xk_t", [E, L], BF16, isOutput=False)
    xv = nc.declare_dram_parameter("xv_t", [E, L], BF16, isOutput=False)
    wq = nc.declare_dram_parameter("wq_t", [E, FL], BF16, isOutput=False)
    wk = nc.declare_dram_parameter("wk_t", [E, FL], BF16, isOutput=False)
    wv = nc.declare_dram_parameter("wv_t", [E, FL], BF16, isOutput=False)
    wo = nc.declare_dram_parameter("wo_t", [FL, E], BF16, isOutput=False)
    bq = nc.declare_dram_parameter("bq", [FL], FP32, isOutput=False)
    bk = nc.declare_dram_parameter("bk", [FL], FP32, isOutput=False)
    bv = nc.declare_dram_parameter("bv", [FL], FP32, isOutput=False)
    out = nc.declare_dram_parameter("out_p", [L, E], BF16, isOutput=True)

    with PatchedTileContext(nc) as tc:
        with (
            tc.tile_pool(name="singles", bufs=1) as singles,
            tc.tile_pool(name="pt", bufs=6) as pt_pool,
            tc.tile_pool(name="norm", bufs=1) as norm_pool,
            tc.tile_pool(name="outsb", bufs=4) as out_pool,
            tc.tile_pool(name="spool", bufs=2, space="PSUM") as spool,
            tc.tile_pool(name="cpool", bufs=2, space="PSUM") as cpool,
        ):
            # ---- persistent SBUF ----
            wq_sb = singles.tile([P, KT, FL], BF16, tag="wq")
            wk_sb = singles.tile([P, KT, FL], BF16, tag="wk")
            wv_sb = singles.tile([P, KT, FL], BF16, tag="wv")
            wo_sb = singles.tile([P, MT, E], BF16, tag="wo")
            bq_sb = singles.tile([P, MT], FP32, tag="bq")
            bk_sb = singles.tile([P, MT], FP32, tag="bk")
            bv_sb = singles.tile([P, FL], FP32, tag="bv")
            qt_sb = singles.tile([P, MT, L], BF16, tag="qt")
            kt_sb = singles.tile([P, MT, L], BF16, tag="kt")
            ctx_sb = singles.tile([P, MT, L], BF16, tag="ctx")
            v_sb = singles.tile([P, LT, HPC, P], BF16, tag="v")
            xq_sb = singles.tile([P, KT, L], BF16, tag="xq")
            xk_sb = singles.tile([P, KT, L], BF16, tag="xk")
            xv_sb = singles.tile([P, KT, L], BF16, tag="xv")

            xq_re = xq.rearrange("(o p) m -> p o m", p=P)
            xk_re = xk.rearrange("(o p) m -> p o m", p=P)
            xv_re = xv.rearrange("(o p) m -> p o m", p=P)

            # ---- DMAs: all on the SP queue (HWDGE), priority ordered; big
            # activations token-quartered so compute starts early.
            nc.sync.dma_start(wq_sb[:], wq.rearrange("(o p) f -> p o f", p=P))
            nc.sync.dma_start(xq_sb[:, :, 0:512], xq_re[:, :, 0:512])
            nc.sync.dma_start(wk_sb[:], wk.rearrange("(o p) f -> p o f", p=P))
            nc.sync.dma_start(xk_sb[:, :, 0:512], xk_re[:, :, 0:512])
            nc.sync.dma_start(bq_sb[:], bq.rearrange("(o p) -> p o", p=P))
            nc.sync.dma_start(bk_sb[:], bk.rearrange("(o p) -> p o", p=P))
            nc.sync.dma_start(bv_sb[:], _bcast_ap(bv))
            nc.sync.dma_start(xq_sb[:, :, 512:1024], xq_re[:, :, 512:1024])
            nc.sync.dma_start(xk_sb[:, :, 512:1024], xk_re[:, :, 512:1024])
            nc.sync.dma_start(wv_sb[:], wv.rearrange("(o p) f -> p o f", p=P))
            nc.sync.dma_start(xv_sb[:, :, 0:512], xv_re[:, :, 0:512])
            nc.sync.dma_start(xv_sb[:, :, 512:1024], xv_re[:, :, 512:1024])
            nc.sync.dma_start(xv_sb[:, :, 1024:1536], xv_re[:, :, 1024:1536])
            nc.sync.dma_start(xv_sb[:, :, 1536:2048], xv_re[:, :, 1536:2048])
            nc.sync.dma_start(xq_sb[:, :, 1024:2048], xq_re[:, :, 1024:2048])
            nc.sync.dma_start(xk_sb[:, :, 1024:2048], xk_re[:, :, 1024:2048])
            nc.sync.dma_start(wo_sb[:], wo.rearrange("(o p) f -> p o f", p=P))

            # V padded layout: [V_h (64) | ones | zeros(63)]
            nc.vector.memset(v_sb[:], 0.0)
            nc.vector.memset(v_sb[:, :, :, D], 1.0)

            # ---------------- emission helpers ----------------
            def proj_quarter(x_sb, w_sb, o_sb, b_sb, mt, nq, ps):
                """One projection quarter: features [128*mt,+128) x tokens
                [512*nq,+512) -> o_sb[:, mt, 512*nq:+512]. ps: [P,1024] psum
                tile; uses column half nq%2."""
                col = (nq % 2) * 512
                for k in range(KT):
                    nc.tensor.matmul(
                        ps[:, col : col + 512],
                        w_sb[:, k, bass.ts(mt, P)],
                        x_sb[:, k, bass.ts(nq, 512)],
                        start=(k == 0),
                        stop=(k == KT - 1),
                    )
                nc.vector.tensor_scalar_add(
                    o_sb[:, mt, bass.ts(nq, 512)],
                    ps[:, col : col + 512],
                    b_sb[:, mt : mt + 1],
                )

            def proj_half(x_sb, w_sb, o_sb, b_sb, mt, nqp, name):
                """Two projection quarters sharing one [P,1024] psum tile."""
                ps = spool.tile([P, 1024], FP32, tag="ps", name=f"pp_{name}")
                proj_quarter(x_sb, w_sb, o_sb, b_sb, mt, 2 * nqp, ps)
                proj_quarter(x_sb, w_sb, o_sb, b_sb, mt, 2 * nqp + 1, ps)

            def proj_q(x_sb, w_sb, o_sb, b_sb, mt, nq, name):
                """One standalone projection quarter (own psum tile)."""
                ps = spool.tile([P, 1024], FP32, tag="ps", name=f"pq_{name}")
                proj_quarter(x_sb, w_sb, o_sb, b_sb, mt, nq, ps)

            def v_group(g):
                """V projection for token tiles lt=4g..4g+3 (one psum tile)."""
                ps = spool.tile([P, 1024], FP32, tag="ps", name=f"vg_{g}")
                for li in range(4):
                    lt = 4 * g + li
                    for k in range(KT):
                        nc.tensor.matmul(
                            ps[:, li * 256 : li * 256 + 256],
                            xv_sb[:, k, bass.ts(lt, P)],
                            wv_sb[:, k, :],
                            start=(k == 0),
                            stop=(k == KT - 1),
                        )
                for li in range(4):
                    lt = 4 * g + li
                    nc.vector.tensor_add(
                        v_sb[:, lt, :, 0:D],
                        ps[:, li * 256 : li * 256 + 256].rearrange(
                            "p (h d) -> p h d", d=D
                        ),
                        bv_sb.rearrange("p (h d) -> p h d", d=D),
                    )

            def c_tile(lt):
                """Out-projection for token tile lt + bias evac + store."""
                ps = spool.tile([P, 1024], FP32, tag="ps", name=f"ot_{lt}")
                for kt in range(MT):
                    for nn in range(2):
                        nc.tensor.matmul(
                            ps[:, bass.ts(nn, 512)],
                            ctx_sb[:, kt, bass.ts(lt, P)],
                            wo_sb[:, kt, bass.ts(nn, 512)],
                            start=(kt == 0),
                            stop=(kt == MT - 1),
                        )
                osb = out_pool.tile([P, E], BF16, tag="osb", name=f"osb_{lt}")
                nc.vector.tensor_copy(osb[:], ps[:])
                nc.sync.dma_start(out[bass.ts(lt, P), :], osb[:])

            # ---------------- attention section ----------------
            def b_section(pair, qh, fill):
                """Attention for head pair `pair` on query half `qh` (1024 q).
                `fill`: dict j -> list of thunks emitted after scores+exp of
                iteration j (PE filler work scheduled into exp-paced slack).
                ctx matmuls lag scores by 2 iterations (pts pool depth 6)."""
                cps = [
                    cpool.tile([P, 1024], FP32, tag="c", name=f"c_{pair}_{qh}_{h}")
                    for h in range(2)
                ]
                pts = {}

                def scores_exp(j):
                    sps = []
                    for hh in range(2):
                        sps.append(
                            spool.tile(
                                [P, 1024], FP32, tag="ps",
                                name=f"s_{pair}_{qh}_{j}_{hh}",
                            )
                        )
                    for nn in range(2):
                        for hh in range(2):
                            r0 = D * hh
                            nc.tensor.matmul(
                                sps[hh][:, bass.ts(nn, 512)],
                                kt_sb[r0 : r0 + D, pair, bass.ts(j, P)],
                                qt_sb[
                                    r0 : r0 + D, pair,
                                    bass.ds(qh * 1024 + nn * 512, 512),
                                ],
                                start=True,
                                stop=True,
                            )
                    pts[j] = []
                    for hh in range(2):
                        pt = pt_pool.tile(
                            [P, 1024], BF16, tag="pt", name=f"pt_{pair}_{qh}_{j}_{hh}"
                        )
                        nc.scalar.activation(
                            pt[:], sps[hh][:], mybir.ActivationFunctionType.Exp
                        )
                        pts[j].append(pt)

                def ctx_mm(j):
                    for hh in range(2):
                        for nn in range(2):
                            nc.tensor.matmul(
                                cps[hh][:, bass.ts(nn, 512)],
                                v_sb[:, j, 2 * pair + hh, :],
                                pts[j][hh][:, bass.ts(nn, 512)],
                                start=(j == 0),
                                stop=(j == LT - 1),
                            )
                    del pts[j]

                LAG = 2
                for j in range(LT):
                    scores_exp(j)
                    for thunk in fill.get(j, ()):
                        thunk()
                    if j >= LAG:
                        ctx_mm(j - LAG)
                for j in range(LT - LAG, LT):
                    ctx_mm(j)

                # normalization: recip of sums row, broadcast, multiply
                for hh in range(2):
                    craw = norm_pool.tile(
                        [D + 1, 1024], FP32, tag="craw", name=f"cr_{pair}_{qh}_{hh}"
                    )
                    nc.vector.tensor_copy(craw[:], cps[hh][0 : D + 1, :])
                    # reciprocal spread over four 32-aligned partitions (a
                    # 1-partition reciprocal is ~6.5 DVE-cycles/element)
                    rt = norm_pool.tile([97, 256], FP32, tag="rt")
                    nc.vector.memset(rt[:], 1.0)
                    for k4 in range(4):
                        nc.vector.tensor_copy(
                            rt[32 * k4 : 32 * k4 + 1, :],
                            craw[D : D + 1, bass.ts(k4, 256)],
                        )
                    nc.vector.reciprocal(rt[:], rt[:])
                    rrow = norm_pool.tile([1, 1024], FP32, tag="rrow")
                    for k4 in range(4):
                        nc.vector.tensor_copy(
                            rrow[0:1, bass.ts(k4, 256)],
                            rt[32 * k4 : 32 * k4 + 1, :],
                        )
                    rb = norm_pool.tile([D, 1024], FP32, tag="rb")
                    rap = rrow[0:1, :]
                    nc.sync.dma_start(
                        out=_ap3(rb[:], [rb[:].ap[0], [1, 1], rb[:].ap[1]]),
                        in_=_ap3(rap, [[1, 1], [0, D], rap.ap[-1]]),
                    )
                    nc.vector.tensor_mul(
                        ctx_sb[D * hh : D * hh + D, pair, bass.ts(qh, 1024)],
                        craw[0:D, :],
                        rb[:],
                    )

            # ---------------- static schedule ----------------
            # Bootstrap: Q then K for pair 0, first query/key half.
            proj_half(xq_sb, wq_sb, qt_sb, bq_sb, 0, 0, "q00")
            proj_half(xk_sb, wk_sb, kt_sb, bk_sb, 0, 0, "k00")

            def pq(x, w, o, b, mt, nq, name):
                return lambda: proj_q(x, w, o, b, mt, nq, name)

            # (p0,qh0): V groups paced ahead of the lagged ctx matmuls;
            # K mt0 tokens 1024:2048 before j=8; Q mt0 cols 1024:2048 for qh1.
            b_section(0, 0, {
                1: [lambda: v_group(0)],
                3: [pq(xk_sb, wk_sb, kt_sb, bk_sb, 0, 2, "k02")],
                4: [pq(xk_sb, wk_sb, kt_sb, bk_sb, 0, 3, "k03")],
                5: [lambda: v_group(1)],
                7: [pq(xq_sb, wq_sb, qt_sb, bq_sb, 0, 2, "q02")],
                # v_group(g) must emit before ctx_mm(4g), i.e. at j <= 4g+2
                9: [lambda: v_group(2)],
                11: [pq(xq_sb, wq_sb, qt_sb, bq_sb, 0, 3, "q03")],
                13: [lambda: v_group(3)],
            })
            # (p0,qh1): fill with pair-1 projections (K first: needed at the
            # start of (p1,qh0); Q mt1 cols 0:1024 too).
            b_section(0, 1, {
                1: [pq(xk_sb, wk_sb, kt_sb, bk_sb, 1, 0, "k10")],
                3: [pq(xk_sb, wk_sb, kt_sb, bk_sb, 1, 1, "k11")],
                5: [pq(xk_sb, wk_sb, kt_sb, bk_sb, 1, 2, "k12")],
                7: [pq(xk_sb, wk_sb, kt_sb, bk_sb, 1, 3, "k13")],
                9: [pq(xq_sb, wq_sb, qt_sb, bq_sb, 1, 0, "q10")],
                11: [pq(xq_sb, wq_sb, qt_sb, bq_sb, 1, 1, "q11")],
                13: [pq(xq_sb, wq_sb, qt_sb, bq_sb, 1, 2, "q12")],
                14: [pq(xq_sb, wq_sb, qt_sb, bq_sb, 1, 3, "q13")],
            })
            b_section(1, 0, {})
            # (p1,qh1): fill with out-projection for the first query half.
            b_section(1, 1, {j: [lambda lt=j - 2: c_tile(lt)] for j in range(2, 10)})
            # tail: out-projection for the second query half.
            for lt in range(8, LT):
                c_tile(lt)

            if DEBUG_DUMP:
                qt_d = nc.declare_dram_parameter("qt_d", [P, MT, L], BF16, isOutput=True)
                kt_d = nc.declare_dram_parameter("kt_d", [P, MT, L], BF16, isOutput=True)
                ctx_d = nc.declare_dram_parameter("ctx_d", [P, MT, L], BF16, isOutput=True)
                v_d = nc.declare_dram_parameter("v_d", [P, LT, HPC, P], BF16, isOutput=True)
                nc.sync.dma_start(qt_d[:], qt_sb[:])
                nc.sync.dma_start(kt_d[:], kt_sb[:])
                nc.sync.dma_start(ctx_d[:], ctx_sb[:])
                nc.sync.dma_start(v_d[:], v_sb[:])

    return nc


_NC = None


def _get_nc():
    global _NC
    if _NC is None:
        _NC = build_nc()
    return _NC


def kernel(query, key, value, w_in, b_in, w_out, b_out):
    import ml_dtypes

    bf16 = ml_dtypes.bfloat16
    query = np.asarray(query, dtype=np.float32)
    key = np.asarray(key, dtype=np.float32)
    value = np.asarray(value, dtype=np.float32)
    w_in = np.asarray(w_in, dtype=np.float32)
    b_in = np.asarray(b_in, dtype=np.float32)
    w_out = np.asarray(w_out, dtype=np.float32)
    b_out = np.asarray(b_out, dtype=np.float32)

    scale = float(D) ** -0.5
    in_maps = []
    for c in range(NCORES):
        b = c % 2
        g = c // 2
        sl = slice(FL * g, FL * (g + 1))
        wq = w_in[0 * E : 1 * E][sl] * scale  # (256, 1024)
        wk = w_in[1 * E : 2 * E][sl]
        wv = w_in[2 * E : 3 * E][sl]
        in_maps.append(
            {
                "xq_t": np.ascontiguousarray(query[:, b, :].T).astype(bf16),
                "xk_t": np.ascontiguousarray(key[:, b, :].T).astype(bf16),
                "xv_t": np.ascontiguousarray(value[:, b, :].T).astype(bf16),
                "wq_t": np.ascontiguousarray(wq.T).astype(bf16),
                "wk_t": np.ascontiguousarray(wk.T).astype(bf16),
                "wv_t": np.ascontiguousarray(wv.T).astype(bf16),
                "wo_t": np.ascontiguousarray(w_out[:, sl].T).astype(bf16),
                "bq": np.ascontiguousarray(b_in[0 * E : 1 * E][sl] * scale),
                "bk": np.ascontiguousarray(b_in[1 * E : 2 * E][sl]),
                "bv": np.ascontiguousarray(b_in[2 * E : 3 * E][sl]),
            }
        )

    nc = _get_nc()
    res = run_bass_kernel_spmd(
        nc, in_maps, list(range(NCORES)), trace=TRACE, **TRACE_KWARGS
    )
    global LAST_RESULTS
    LAST_RESULTS = res

    out = np.zeros((L, B, E), dtype=np.float32)
    for c in range(NCORES):
        out[:, c % 2, :] += res.results[c]["out_p"].astype(np.float32)
    out += b_out
    return out
